# revision 17
# baseline (speedup 1.0000x reference)
"""nn_Block_67173288509603 on 8 TRN2 NeuronCores via Bass/Tile.

adaLN -> GQA block-causal attention (+RoPE) -> adaLN -> MoE (shared + top2-of-8).

v3: bf16 matmul paths, slab-pipelined chunked collectives, batch-local MoE.

Sharding (single SPMD program; per-core differences flow through inputs and
replica-group semantics):
  core c in 0..7, b = c//4 (batch), g = c%4 (kv-head group),
  experts {2g, 2g+1} for batch b, shared-expert hidden slice g of 4.
  Token ownership is INTERLEAVED: core c owns token tiles {4j+g : j=0..3} of
  batch b (tile = 128 tokens).  Slab j = global tokens [j*512,(j+1)*512).

  - Phase A: ada GEMV (fp32, 4-way sharded + tiny AllGather; scale/shift kept
    as per-C-dim columns), LN1 stats, per-owned-tile transpose of the
    normalized x with fused scale/shift applied on the transposed tiles,
    4 chunked AllGathers (bf16) of h1^T.
  - Phase B (per slab j): QKV projection (bf16) + RoPE (rot-half via +-1
    permutation matmul), block-causal attention for 4 q-heads/1 kv-head in
    bf16 (exp trick, fused denominator row), output projection partial,
    chunked ReduceScatter (own tile) + AllGather (full slab) over the batch
    group, both bf16.
  - Phase C (per slab j): own-tile residual for the final output; uniformly
    recompute x2/LN2 for all 4 tiles of the slab from x_full + gathered
    attention (bit-identical across the batch group), transpose into local
    h2^T (bf16), fp32 router logits + exact top-2 gates; per-core expert-pair
    gate rows selected by a one-hot matmul, bounced via DRAM for broadcast.
    No collectives.
  - Phase D (per chunk j = slab j): shared expert (hidden 1/4) + expert pair
    {2g, 2g+1} dense-masked, fused PSUM accumulation, chunked ReduceScatter
    over the batch group, final residual.
Output: each core returns its [512, 1024] interleaved quarter; host scatters.
"""

import os
import numpy as np
import ml_dtypes

import concourse.bass as bass
import concourse.mybir as mybir
import concourse.tile as tile
from concourse import bacc
from concourse.bass_utils import run_bass_kernel_spmd

F32 = mybir.dt.float32
BF16 = mybir.dt.bfloat16
AX = mybir.AxisListType
OP = mybir.AluOpType
ACT = mybir.ActivationFunctionType

B, T, C = 2, 2048, 1024
H, KVH, HD = 16, 4, 64
BLK = 128
THETA = 10000.0
E, TOPK = 8, 2
EPS_LN = 1e-5
P = 128
NCORE = 8
TQ = 512              # tokens per core quarter / slab size
NT_Q = TQ // P        # 4 owned tiles / slabs
CK = C // P           # 8 contraction tiles over C
NKB = T // BLK        # 16 kv blocks

LAST_EXEC_NS = None

GROUPS_B = [[0, 1, 2, 3], [4, 5, 6, 7]]


def build_program():
    nc = bacc.Bacc("TRN2", target_bir_lowering=False, debug=False,
                   num_devices=NCORE)

    def din(name, shape, dt):
        return nc.dram_tensor(name, list(shape), dt, kind="ExternalInput").ap()

    io = dict(
        x_q=din("x_q", [TQ, C], F32),
        x_full=din("x_full", [T, C], F32),
        temb_b=din("temb_b", [C, 1], F32),
        ada_w_s=din("ada_w_s", [C, 1024], F32),
        ada_b_s=din("ada_b_s", [1, 1024], F32),
        wq_s=din("wq_s", [C, 256], BF16),
        wkv_s=din("wkv_s", [C, 128], BF16),
        wo_s=din("wo_s", [256, C], BF16),
        cosq=din("cosq", [64, T], F32),
        sinq=din("sinq", [64, T], F32),
        identf=din("identf", [P, P], F32),
        rotp=din("rotp", [64, 64], BF16),
        swA_s=din("swA_s", [C, 1024], BF16),
        sw2_s=din("sw2_s", [512, C], BF16),
        rw1_e=din("rw1_e", [C, 2048], BF16),
        rw2_e=din("rw2_e", [2048, C], BF16),
        router_w=din("router_w", [C, E], F32),
        router_bias=din("router_bias", [1, E], F32),
        esel2=din("esel2", [E, 2], F32),
        out=nc.dram_tensor("out", [TQ, C], F32, kind="ExternalOutput").ap(),
    )

    with tile.TileContext(nc) as tc:
        _build(tc, io)
    nc.compile()
    return nc


def _build(tc, io):
    nc = tc.nc
    from contextlib import ExitStack
    PH = os.environ.get("KB_PHASES", "FULL")
    F_TTR = os.environ.get("KB_TTR", "0") == "1"  # InstTensorTensorReduce hangs this device
    F_VRECIP = os.environ.get("KB_VRECIP", "1") == "1"
    F_TS2 = os.environ.get("KB_TS2", "1") == "1"

    top = ExitStack()
    with top:
        dram = top.enter_context(tc.tile_pool(name="dram", bufs=1, space="DRAM"))
        pers = top.enter_context(tc.tile_pool(name="pers", bufs=1))

        # ---- chunked collective + scratch DRAM buffers --------------------
        ag_ada_in = dram.tile([1, 1024], F32, name="ag_ada_in")
        ag_ada_out = dram.tile([4, 1024], F32, name="ag_ada_out")
        h1ag_in = [dram.tile([C, P], BF16, name=f"h1ag_in_{j}")
                   for j in range(NT_Q)]
        h1ag_out = [dram.tile([4 * C, P], BF16, name=f"h1ag_out_{j}")
                    for j in range(NT_Q)]
        rsat_in = [dram.tile([TQ, C], BF16, name=f"rsat_in_{j}")
                   for j in range(NT_Q)]
        rsat_out = [dram.tile([P, C], BF16, name=f"rsat_out_{j}")
                    for j in range(NT_Q)]
        atag_in = [dram.tile([P, C], BF16, name=f"atag_in_{j}")
                   for j in range(NT_Q)]
        atag_out = [dram.tile([TQ, C], BF16, name=f"atag_out_{j}")
                    for j in range(NT_Q)]
        g2 = [dram.tile([2, TQ], BF16, name=f"g2_{j}") for j in range(NT_Q)]
        rsmo_in = [dram.tile([TQ, C], BF16, name=f"rsmo_in_{j}")
                   for j in range(NT_Q)]
        rsmo_out = [dram.tile([P, C], BF16, name=f"rsmo_out_{j}")
                    for j in range(NT_Q)]

        # ---- whole-kernel persistents (loads first: startup critical) -----
        x2 = [pers.tile([P, C], F32, name=f"x2_{t}", tag=f"x2_{t}")
              for t in range(NT_Q)]
        for t in range(NT_Q):
            nc.sync.dma_start(x2[t][:], io["x_q"][t * P:(t + 1) * P, :])
        identf = pers.tile([P, P], F32, name="identf", tag="identf")
        nc.sync.dma_start(identf[:], io["identf"][:])
        rotp = pers.tile([64, 64], BF16, name="rotp", tag="rotp")
        nc.sync.dma_start(rotp[:], io["rotp"][:])
        # ada scale/shift as per-C-dim columns: [P, 32], col a*8+k
        scsh = pers.tile([P, 32], F32, name="scsh", tag="scsh")
        ones1 = pers.tile([1, 64], BF16, name="ones1", tag="ones1")
        nc.vector.memset(ones1[:], 1.0)
        zl = pers.tile([1, 65], BF16, name="zl", tag="zl")
        nc.vector.memset(zl[:], 0.0)
        zr = pers.tile([1, 512], BF16, name="zr", tag="zr")
        nc.vector.memset(zr[:], 0.0)
        bias_bc = pers.tile([P, E], F32, name="bias_bc", tag="bias_bc")
        nc.sync.dma_start(
            bias_bc[:], io["router_bias"][0, :][None, :].to_broadcast([P, E]))
        esel2 = pers.tile([E, 2], F32, name="esel2", tag="esel2")
        nc.sync.dma_start(esel2[:], io["esel2"][:])
        rtw = [pers.tile([P, E], F32, name=f"rtw_{k}", tag=f"rtw{k}")
               for k in range(CK)]
        for k in range(CK):
            nc.sync.dma_start(rtw[k][:], io["router_w"][k * P:(k + 1) * P, :])

        # =====================================================================
        # Phase A
        # =====================================================================
        pa_stack = ExitStack()
        pa = pa_stack.enter_context(tc.tile_pool(name="pa", bufs=1))
        pa_ps = pa_stack.enter_context(tc.tile_pool(name="pa_ps", bufs=2,
                                                    space="PSUM"))

        # ada GEMV (fp32, own 1024-slice) + AllGather over batch group
        temb_sb = []
        adaw_sb = []
        for k in range(CK):
            tt = pa.tile([P, 1], F32, name=f"temb_{k}", tag=f"temb{k}")
            nc.sync.dma_start(tt[:], io["temb_b"][k * P:(k + 1) * P, :])
            temb_sb.append(tt)
            wt = pa.tile([P, 1024], F32, name=f"adaw_{k}", tag=f"adaw{k}")
            nc.sync.dma_start(wt[:], io["ada_w_s"][k * P:(k + 1) * P, :])
            adaw_sb.append(wt)
        adab_sb = pa.tile([1, 1024], F32, name="adab_sb", tag="adab_sb")
        nc.sync.dma_start(adab_sb[:], io["ada_b_s"][:])
        ada_sb = pa.tile([1, 1024], F32, name="ada_sb", tag="ada_sb")
        for n in range(2):
            ps = pa_ps.tile([1, 512], F32, name="ada_ps", tag="ada_ps", bufs=2)
            for k in range(CK):
                nc.tensor.matmul(ps[:], temb_sb[k][:],
                                 adaw_sb[k][:, n * 512:(n + 1) * 512],
                                 start=(k == 0), stop=(k == CK - 1))
            nc.vector.tensor_add(ada_sb[:, n * 512:(n + 1) * 512], ps[:],
                                 adab_sb[:, n * 512:(n + 1) * 512])
        nc.sync.dma_start(ag_ada_in[:], ada_sb[:])
        nc.gpsimd.collective_compute(
            "AllGather", OP.bypass, replica_groups=GROUPS_B,
            ins=[ag_ada_in.opt()], outs=[ag_ada_out.opt()])
        nc.sync.dma_start(
            scsh[:], ag_ada_out[:].rearrange("a (k p) -> p (a k)", p=P))
        nc.vector.tensor_scalar_add(scsh[:, 0:8], scsh[:, 0:8], 1.0)
        nc.vector.tensor_scalar_add(scsh[:, 16:24], scsh[:, 16:24], 1.0)
        sc1c, sh1c = scsh[:, 0:8], scsh[:, 8:16]
        sc2c, sh2c = scsh[:, 16:24], scsh[:, 24:32]

        def ln_stats(pool, x_sb, var4, mean4, col, name):
            """mean/var stats of x_sb into column col of packed [P,4] tiles.

            var = E[x^2] - mean^2 (uncentered; fine for |x|~1, mean~0.03)."""
            s1 = pool.tile([P, 1], F32, name=f"{name}_s1", tag="ln_s1", bufs=2)
            nc.vector.tensor_reduce(s1[:], x_sb[:], axis=AX.X, op=OP.add)
            nc.vector.tensor_scalar_mul(mean4[:, col:col + 1], s1[:], 1.0 / C)
            sq = pool.tile([P, C], F32, name=f"{name}_sq", tag="ln_sq", bufs=1)
            ssq = pool.tile([P, 1], F32, name=f"{name}_ssq", tag="ln_ssq",
                            bufs=2)
            if F_TTR:
                nc.vector.tensor_tensor_reduce(sq[:], x_sb[:], x_sb[:], 1.0,
                                               0.0, op0=OP.mult, op1=OP.add,
                                               accum_out=ssq[:])
            else:
                nc.scalar.activation(sq[:], x_sb[:], ACT.Square,
                                     accum_out=ssq[:])
            nc.vector.tensor_scalar(var4[:, col:col + 1], ssq[:], 1.0 / C,
                                    EPS_LN, op0=OP.mult, op1=OP.add)

        def ln_finish4(pool, var4, mean4, name):
            """rstd4 = 1/sqrt(var4 - mean4^2); mrs4 = mean4*rstd4."""
            m2 = pool.tile([P, 4], F32, name=f"{name}_m2", tag="ln_m2",
                           bufs=2)
            nc.vector.tensor_mul(m2[:], mean4[:], mean4[:])
            nc.vector.tensor_sub(var4[:], var4[:], m2[:])
            rstd4 = pool.tile([P, 4], F32, name=f"{name}_rstd", tag="ln_rstd",
                              bufs=2)
            if F_VRECIP:
                std4 = pool.tile([P, 4], F32, name=f"{name}_std",
                                 tag="ln_std", bufs=2)
                nc.scalar.activation(std4[:], var4[:], ACT.Sqrt)
                nc.vector.reciprocal(rstd4[:], std4[:])
            else:
                t4 = pool.tile([P, 4], F32, name=f"{name}_lnr", tag="ln_std",
                               bufs=2)
                nc.scalar.activation(t4[:], var4[:], ACT.Ln)
                nc.scalar.activation(rstd4[:], t4[:], ACT.Exp, scale=-0.5)
            mrs4 = pool.tile([P, 4], F32, name=f"{name}_mrs", tag="ln_mrs",
                             bufs=2)
            nc.vector.tensor_mul(mrs4[:], mean4[:], rstd4[:])
            return rstd4, mrs4

        # LN1: packed stats for all tiles (one Sqrt table load), then
        # normalize + transpose + scale/shift on transposed tiles + AG chunks
        var4a = pa.tile([P, 4], F32, name="var4a", tag="ln_v4")
        mean4a = pa.tile([P, 4], F32, name="mean4a", tag="ln_mn4")
        for t in range(NT_Q):
            ln_stats(pa, x2[t], var4a, mean4a, t, f"l1_{t}")
        rstd1, mrs1 = ln_finish4(pa, var4a, mean4a, "l1")
        for t in range(NT_Q):
            hn = pa.tile([P, C], F32, name=f"hn_{t}", tag="hn", bufs=2)
            if F_TS2:
                nc.vector.tensor_scalar(hn[:], x2[t][:], rstd1[:, t:t + 1],
                                        mrs1[:, t:t + 1],
                                        op0=OP.mult, op1=OP.subtract)
            else:
                nc.vector.tensor_scalar_mul(hn[:], x2[t][:], rstd1[:, t:t + 1])
                nc.vector.tensor_scalar_sub(hn[:], hn[:], mrs1[:, t:t + 1])
            for k in range(CK):
                tp = pa_ps.tile([P, P], F32, name="h1t_ps", tag="h1t_ps",
                                bufs=2)
                nc.tensor.transpose(tp[:], hn[:, k * P:(k + 1) * P], identf[:])
                stg = pa.tile([P, P], BF16, name=f"h1t_{t}_{k}", tag="h1t",
                              bufs=3)
                if F_TS2:
                    nc.vector.tensor_scalar(stg[:], tp[:], sc1c[:, k:k + 1],
                                            sh1c[:, k:k + 1],
                                            op0=OP.mult, op1=OP.add)
                else:
                    st32 = pa.tile([P, P], F32, name=f"h1s_{t}_{k}",
                                   tag="h1s32", bufs=2)
                    nc.vector.tensor_scalar_mul(st32[:], tp[:],
                                                sc1c[:, k:k + 1])
                    nc.vector.tensor_scalar(stg[:], st32[:], sh1c[:, k:k + 1],
                                            None, op0=OP.add)
                nc.sync.dma_start(h1ag_in[t][k * P:(k + 1) * P, :], stg[:])
            nc.gpsimd.collective_compute(
                "AllGather", OP.bypass, replica_groups=GROUPS_B,
                ins=[h1ag_in[t].opt()], outs=[h1ag_out[t].opt()])
        pa_stack.close()

        def dummy_out():
            with tc.tile_pool(name="dz", bufs=1) as pz:
                for t in range(NT_Q):
                    z = pz.tile([P, C], F32, name=f"dz_{t}", tag="dz", bufs=2)
                    nc.vector.memset(z[:], 0.0)
                    nc.sync.dma_start(io["out"][t * P:(t + 1) * P, :], z[:])
        if PH == "A":
            dummy_out()
            return

        # =====================================================================
        # pools for B/C/D
        # =====================================================================
        pc = top.enter_context(tc.tile_pool(name="pc", bufs=1))
        pc_ps = top.enter_context(tc.tile_pool(name="pc_ps", bufs=2,
                                               space="PSUM"))
        ph2 = top.enter_context(tc.tile_pool(name="ph2", bufs=1))
        h2T = {}  # (j, k) -> tile [P, TQ] bf16, local h2^T columns of slab j

        def phase_c(j):
            """own-tile residual; uniform slab LN2 -> h2T + gates (no comms)."""
            at_bf = pc.tile([P, C], BF16, name=f"at_{j}", tag="at", bufs=2)
            nc.sync.dma_start(at_bf[:], rsat_out[j][:])
            nc.vector.tensor_add(x2[j][:], x2[j][:], at_bf[:])
            # uniform path over the slab's 4 tiles
            var4 = pc.tile([P, 4], F32, name=f"var4_{j}", tag="ln_v4",
                           bufs=2)
            mean4 = pc.tile([P, 4], F32, name=f"mean4_{j}", tag="ln_mn4",
                            bufs=2)
            xfs = []
            for r in range(4):
                row = j * TQ + r * P
                xf = pc.tile([P, C], F32, name=f"xf_{j}_{r}", tag="xf",
                             bufs=4)
                nc.sync.dma_start(xf[:], io["x_full"][row:row + P, :])
                ab = pc.tile([P, C], BF16, name=f"ab_{j}_{r}", tag="ab",
                             bufs=2)
                nc.sync.dma_start(ab[:], atag_out[j][r * P:(r + 1) * P, :])
                nc.vector.tensor_add(xf[:], xf[:], ab[:])
                ln_stats(pc, xf, var4, mean4, r, f"l2_{j}_{r}")
                xfs.append(xf)
            rstd4, mrs4 = ln_finish4(pc, var4, mean4, f"l2_{j}")
            lgs_all = []
            for r in range(4):
                hn = pc.tile([P, C], F32, name=f"hn2_{j}_{r}", tag="hn2",
                             bufs=1)
                nc.vector.tensor_scalar(hn[:], xfs[r][:], rstd4[:, r:r + 1],
                                        mrs4[:, r:r + 1],
                                        op0=OP.mult, op1=OP.subtract)
                lg_sb = pc.tile([E, P], F32, name=f"lg_{j}_{r}", tag=f"lg{r}",
                                bufs=1)
                for k in range(CK):
                    tp = pc_ps.tile([P, P], F32, name="h2t_ps", tag="c_ps",
                                    bufs=2)
                    nc.tensor.transpose(tp[:], hn[:, k * P:(k + 1) * P],
                                        identf[:])
                    s32 = pc.tile([P, P], F32, name=f"h2s_{j}_{r}_{k}",
                                  tag="h2s32", bufs=2)
                    nc.vector.tensor_scalar(s32[:], tp[:], sc2c[:, k:k + 1],
                                            sh2c[:, k:k + 1],
                                            op0=OP.mult, op1=OP.add)
                    nc.vector.tensor_scalar(
                        h2T[(j, k)][:, r * P:(r + 1) * P], tp[:],
                        sc2c[:, k:k + 1], sh2c[:, k:k + 1],
                        op0=OP.mult, op1=OP.add)
                    lgp = pc_ps.tile([P, P], F32, name="lgp_ps", tag="c_ps",
                                     bufs=2)
                    nc.tensor.matmul(lgp[0:E, :], rtw[k][:], s32[:],
                                     start=True, stop=True)
                    lgs = pc.tile([E, P], F32, name="lgs", tag="lgs", bufs=2)
                    nc.vector.tensor_copy(lgs[:], lgp[0:E, :])
                    if k == 0:
                        nc.vector.tensor_copy(lg_sb[:], lgs[:])
                    else:
                        nc.vector.tensor_add(lg_sb[:], lg_sb[:], lgs[:])
                lgs_all.append(lg_sb)
            # gates for the 4 tiles (batched sigmoid)
            sels = []
            for r in range(4):
                tp2 = pc_ps.tile([P, P], F32, name="lgT_ps", tag="c_ps",
                                 bufs=2)
                nc.tensor.transpose(tp2[:, 0:E], lgs_all[r][:],
                                    identf[0:E, 0:E])
                s_sb = pc.tile([P, E], F32, name=f"s_{j}_{r}", tag=f"s{r}",
                               bufs=1)
                nc.scalar.activation(s_sb[:], tp2[:, 0:E], ACT.Sigmoid)
                sels.append(s_sb)
            for r in range(4):
                s_sb = sels[r]
                sel = pc.tile([P, E], F32, name=f"sel_{j}_{r}", tag="sel",
                              bufs=2)
                nc.vector.tensor_add(sel[:], s_sb[:], bias_bc[:])
                m8 = pc.tile([P, 8], F32, name=f"m8_{j}_{r}", tag="m8",
                             bufs=2)
                nc.vector.max(m8[:], sel[:])
                mask = pc.tile([P, E], F32, name=f"mask_{j}_{r}", tag="mask",
                               bufs=2)
                nc.vector.tensor_scalar(mask[:], sel[:], m8[:, 1:2], None,
                                        op0=OP.is_ge)
                sm = pc.tile([P, E], F32, name=f"sm_{j}_{r}", tag="sm",
                             bufs=2)
                nc.vector.tensor_mul(sm[:], s_sb[:], mask[:])
                den = pc.tile([P, 1], F32, name=f"den_{j}_{r}", tag="den",
                              bufs=2)
                nc.vector.tensor_reduce(den[:], sm[:], axis=AX.X, op=OP.add)
                nc.vector.tensor_scalar_add(den[:], den[:], 1e-9)
                rden = pc.tile([P, 1], F32, name=f"rden_{j}_{r}", tag="rden",
                               bufs=2)
                nc.vector.reciprocal(rden[:], den[:])
                ge = pc.tile([P, E], F32, name=f"ge_{j}_{r}", tag="ge",
                             bufs=2)
                nc.vector.tensor_scalar_mul(ge[:], sm[:], rden[:])
                gT_ps = pc_ps.tile([P, P], F32, name="gT_ps", tag="c_ps",
                                   bufs=2)
                nc.tensor.transpose(gT_ps[0:E, :], ge[:], identf[:])
                geT = pc.tile([E, P], F32, name=f"geT_{j}_{r}", tag="geT",
                              bufs=2)
                nc.vector.tensor_copy(geT[:], gT_ps[0:E, :])
                sel_ps = pc_ps.tile([P, P], F32, name="sel_ps", tag="c_ps",
                                    bufs=2)
                nc.tensor.matmul(sel_ps[0:2, :], esel2[:], geT[:],
                                 start=True, stop=True)
                sel_bf = pc.tile([2, P], BF16, name=f"selb_{j}_{r}",
                                 tag="selb", bufs=2)
                nc.vector.tensor_copy(sel_bf[:], sel_ps[0:2, :])
                nc.sync.dma_start(g2[j][:, r * P:(r + 1) * P], sel_bf[:])

        # ---------------- Phase B ----------------
        pb = ExitStack()
        pB = pb.enter_context(tc.tile_pool(name="pB", bufs=1))
        pb_mm = pb.enter_context(tc.tile_pool(name="pb_mm", bufs=2,
                                              space="PSUM"))
        pb_st = pb.enter_context(tc.tile_pool(name="pb_st", bufs=2,
                                              space="PSUM"))
        pb_yt = pb.enter_context(tc.tile_pool(name="pb_yt", bufs=2,
                                              space="PSUM"))
        cosq = pB.tile([64, T], F32, name="cosq", tag="cosq")
        sinq = pB.tile([64, T], F32, name="sinq", tag="sinq")
        nc.sync.dma_start(cosq[:], io["cosq"][:])
        nc.sync.dma_start(sinq[:], io["sinq"][:])
        wq_sb, wkv_sb = [], []
        for k in range(CK):
            w = pB.tile([P, 256], BF16, name=f"wq_{k}", tag=f"wq{k}")
            nc.sync.dma_start(w[:], io["wq_s"][k * P:(k + 1) * P, :])
            wq_sb.append(w)
            w = pB.tile([P, P], BF16, name=f"wkv_{k}", tag=f"wkv{k}")
            nc.sync.dma_start(w[:], io["wkv_s"][k * P:(k + 1) * P, :])
            wkv_sb.append(w)
        wo_sb = []
        for k in range(2):
            w = pB.tile([P, C], BF16, name=f"wo_{k}", tag=f"wo{k}")
            nc.sync.dma_start(w[:], io["wo_s"][k * P:(k + 1) * P, :])
            wo_sb.append(w)
        krope = pB.tile([64, T], BF16, name="krope", tag="krope")
        vav = [pB.tile([P, 65], BF16, name=f"vav_{kb}", tag=f"vav{kb}")
               for kb in range(NKB)]
        for kb in range(NKB):
            nc.vector.memset(vav[kb][:, 64:65], 1.0)

        def slab_b(j):
            js = j * 512
            h1c = []
            for k in range(CK):
                hc = pB.tile([P, 512], BF16, name=f"h1c_{j}_{k}",
                             tag=f"h1c{k}", bufs=2)
                for r in range(4):
                    nc.sync.dma_start(
                        hc[:, r * P:(r + 1) * P],
                        h1ag_out[j][r * C + k * P: r * C + (k + 1) * P, :])
                h1c.append(hc)
            # K projection + rope
            kv_ps = pb_mm.tile([P, 512], F32, name="kv_ps", tag="mm", bufs=2)
            for k in range(CK):
                nc.tensor.matmul(kv_ps[0:64, :], wkv_sb[k][:, 0:64],
                                 h1c[k][:], start=(k == 0), stop=(k == CK - 1))
            kt = pB.tile([64, 512], BF16, name="kt", tag="kt", bufs=2)
            nc.vector.tensor_copy(kt[:], kv_ps[0:64, :])
            krot_ps = pb_mm.tile([P, 512], F32, name="krot_ps", tag="mm",
                                 bufs=2)
            nc.tensor.matmul(krot_ps[0:64, :], rotp[:], kt[:],
                             start=True, stop=True)
            kc = pB.tile([64, 512], F32, name="kc", tag="kc", bufs=1)
            nc.vector.tensor_mul(kc[:], kt[:], cosq[:, js:js + 512])
            ks = pB.tile([64, 512], F32, name="ks", tag="ks", bufs=1)
            nc.vector.tensor_mul(ks[:], krot_ps[0:64, :],
                                 sinq[:, js:js + 512])
            nc.vector.tensor_add(krope[:, js:js + 512], kc[:], ks[:])
            # V^T directly: vav[4j+i][:, 0:64] = (h1c_block)^T @ wv
            for i in range(4):
                vav_ps = pb_mm.tile([P, 512], F32, name="vav_ps", tag="mm",
                                    bufs=2)
                for k in range(CK):
                    nc.tensor.matmul(vav_ps[:, 0:64],
                                     h1c[k][:, i * P:(i + 1) * P],
                                     wkv_sb[k][:, 64:128],
                                     start=(k == 0), stop=(k == CK - 1))
                nc.vector.tensor_copy(vav[4 * j + i][:, 0:64],
                                      vav_ps[:, 0:64])
            # Q projection + rope (4 heads)
            qrope = []
            for h in range(4):
                q_ps = pb_mm.tile([P, 512], F32, name="q_ps", tag="mm",
                                  bufs=2)
                for k in range(CK):
                    nc.tensor.matmul(q_ps[0:64, :],
                                     wq_sb[k][:, h * 64:(h + 1) * 64],
                                     h1c[k][:], start=(k == 0),
                                     stop=(k == CK - 1))
                qt = pB.tile([64, 512], BF16, name="qt", tag="qt", bufs=3)
                nc.vector.tensor_copy(qt[:], q_ps[0:64, :])
                qrot_ps = pb_mm.tile([P, 512], F32, name="qrot_ps", tag="mm",
                                     bufs=2)
                nc.tensor.matmul(qrot_ps[0:64, :], rotp[:], qt[:],
                                 start=True, stop=True)
                qc = pB.tile([64, 512], F32, name="qc", tag="qc", bufs=1)
                nc.vector.tensor_mul(qc[:], qt[:], cosq[:, js:js + 512])
                qs = pB.tile([64, 512], F32, name="qs", tag="qs", bufs=1)
                nc.vector.tensor_mul(qs[:], qrot_ps[0:64, :],
                                     sinq[:, js:js + 512])
                qr = pB.tile([64, 512], BF16, name=f"qr_{h}", tag=f"qr{h}",
                             bufs=2)
                nc.vector.tensor_add(qr[:], qc[:], qs[:])
                qrope.append(qr)
            # attention per head
            ytall = [pB.tile([P, 512], BF16, name=f"ytall_{j}_{i}",
                             tag=f"ytall{i}", bufs=2) for i in range(2)]
            for h in range(4):
                yt = pb_yt.tile([65, 512], F32, name="yt_ps", tag="yt",
                                bufs=2)
                for kb in range(4 * j + 4):
                    qoff = max(0, (kb - 4 * j) * P)
                    w = 512 - qoff
                    st = pb_st.tile([P, 512], F32, name="st_ps", tag="st",
                                    bufs=2)
                    nc.tensor.matmul(st[:, :w],
                                     krope[:, kb * P:(kb + 1) * P],
                                     qrope[h][:, qoff:512],
                                     start=True, stop=True)
                    pexp = pB.tile([P, 512], BF16, name="pexp", tag="pexp",
                                   bufs=4)
                    nc.scalar.activation(pexp[:, :w], st[:, :w], ACT.Exp,
                                         scale=0.125)
                    nc.tensor.matmul(yt[:, qoff:512], vav[kb][:],
                                     pexp[:, :w], start=(kb == 0),
                                     stop=False)
                nc.tensor.matmul(yt[:], zl[:], zr[:], start=False, stop=True)
                lrow = pB.tile([1, 512], F32, name="lrow", tag="lrow", bufs=1)
                nc.vector.tensor_copy(lrow[:], yt[64:65, :])
                rec = pB.tile([1, 512], F32, name="rec", tag="rec", bufs=1)
                nc.vector.reciprocal(rec[:], lrow[:])
                rec_bf = pB.tile([1, 512], BF16, name="rec_bf", tag="rec_bf",
                                 bufs=2)
                nc.vector.tensor_copy(rec_bf[:], rec[:])
                recb = pb_mm.tile([P, 512], F32, name="recb_ps", tag="mm",
                                  bufs=2)
                nc.tensor.matmul(recb[0:64, :], ones1[:], rec_bf[:],
                                 start=True, stop=True)
                ytc = pB.tile([64, 512], BF16, name="ytc", tag="ytc", bufs=2)
                nc.vector.tensor_copy(ytc[:], yt[0:64, :])
                nc.vector.tensor_mul(
                    ytall[h // 2][(h % 2) * 64:(h % 2) * 64 + 64, :],
                    ytc[:], recb[0:64, :])
            # output projection + RS (own tile) + AG (full slab)
            for tt in range(4):
                for n in range(2):
                    o_ps = pb_mm.tile([P, 512], F32, name="o_ps", tag="mm",
                                      bufs=2)
                    for k in range(2):
                        nc.tensor.matmul(o_ps[:],
                                         ytall[k][:, tt * P:(tt + 1) * P],
                                         wo_sb[k][:, n * 512:(n + 1) * 512],
                                         start=(k == 0), stop=(k == 1))
                    o_bf = pB.tile([P, 512], BF16, name="o_bf", tag="o_bf",
                                   bufs=3)
                    nc.vector.tensor_copy(o_bf[:], o_ps[:])
                    nc.sync.dma_start(
                        rsat_in[j][tt * P:(tt + 1) * P,
                                   n * 512:(n + 1) * 512], o_bf[:])
            nc.gpsimd.collective_compute(
                "ReduceScatter", OP.add, replica_groups=GROUPS_B,
                ins=[rsat_in[j].opt()], outs=[rsat_out[j].opt()])
            # bounce the RS output through SBUF into a fresh buffer: an AG
            # whose input aliases another collective's output lacks the
            # input-writer sync annotation and hangs the device.
            atb = pB.tile([P, C], BF16, name=f"atb_{j}", tag="atb", bufs=2)
            nc.sync.dma_start(atb[:], rsat_out[j][:])
            nc.sync.dma_start(atag_in[j][:], atb[:])
            nc.gpsimd.collective_compute(
                "AllGather", OP.bypass, replica_groups=GROUPS_B,
                ins=[atag_in[j].opt()], outs=[atag_out[j].opt()])

        def alloc_h2T(j):
            for k in range(CK):
                h2T[(j, k)] = ph2.tile([P, TQ], BF16, name=f"h2T_{j}_{k}",
                                       tag=f"h2T{k}", bufs=2)

        if PH in ("B", "C"):
            slab_b(0)
            slab_b(1)
            slab_b(2)
            slab_b(3)
            pb.close()
            if PH == "B":
                dummy_out()
                return
            for j in range(NT_Q):
                alloc_h2T(j)
                phase_c(j)
            dummy_out()
            return
        slab_b(0)
        slab_b(1)
        alloc_h2T(0)
        phase_c(0)
        slab_b(2)
        alloc_h2T(1)
        phase_c(1)
        slab_b(3)
        pb.close()

        # ---------------- Phase D ----------------
        prw = top.enter_context(tc.tile_pool(name="prw", bufs=1))
        pd = top.enter_context(tc.tile_pool(name="pd", bufs=1))
        pd_ps = top.enter_context(tc.tile_pool(name="pd_ps", bufs=3,
                                               space="PSUM"))
        swA_sb, rw1_sb = [], []
        for k in range(CK):
            w = prw.tile([P, 1024], BF16, name=f"swA_{k}", tag=f"swA{k}")
            nc.sync.dma_start(w[:], io["swA_s"][k * P:(k + 1) * P, :])
            swA_sb.append(w)
            w = prw.tile([P, 2048], BF16, name=f"rw1_{k}", tag=f"rw1{k}")
            nc.sync.dma_start(w[:], io["rw1_e"][k * P:(k + 1) * P, :])
            rw1_sb.append(w)
        sw2_sb = []
        for k in range(4):
            w = prw.tile([P, C], BF16, name=f"sw2_{k}", tag=f"sw2{k}")
            nc.sync.dma_start(w[:], io["sw2_s"][k * P:(k + 1) * P, :])
            sw2_sb.append(w)
        rw2_sb = []
        for k in range(16):
            w = prw.tile([P, C], BF16, name=f"rw2_{k}", tag=f"rw2{k}")
            nc.sync.dma_start(w[:], io["rw2_e"][k * P:(k + 1) * P, :])
            rw2_sb.append(w)

        def chunk_d(j):
            gb = []
            for e in range(2):
                g = pd.tile([P, TQ], BF16, name=f"gb_{j}_{e}", tag=f"gb{e}",
                            bufs=1)
                nc.sync.dma_start(
                    g[:], g2[j][e, :][None, :].to_broadcast([P, TQ]))
                gb.append(g)
            # shared expert (hidden slice g of 4 -> 512 dims local)
            hTs = []
            for m in range(4):
                a1 = pd_ps.tile([P, 512], F32, name="a1_ps", tag="d_ps",
                                bufs=3)
                for k in range(CK):
                    nc.tensor.matmul(a1[:], swA_sb[k][:, m * P:(m + 1) * P],
                                     h2T[(j, k)][:],
                                     start=(k == 0), stop=(k == CK - 1))
                stmp = pd.tile([P, 512], BF16, name="stmp", tag="stmp",
                               bufs=2)
                nc.scalar.activation(stmp[:], a1[:], ACT.Silu)
                a3 = pd_ps.tile([P, 512], F32, name="a3_ps", tag="d_ps",
                                bufs=3)
                for k in range(CK):
                    nc.tensor.matmul(
                        a3[:], swA_sb[k][:, 512 + m * P:512 + (m + 1) * P],
                        h2T[(j, k)][:], start=(k == 0), stop=(k == CK - 1))
                ht = pd.tile([P, 512], BF16, name=f"hTs_{j}_{m}",
                             tag=f"hTs{m}", bufs=1)
                nc.vector.tensor_mul(ht[:], stmp[:], a3[:])
                hTs.append(ht)
            # routed expert pair
            hmid = []
            for e in range(2):
                for m in range(CK):
                    w1 = pd_ps.tile([P, 512], F32, name="w1_ps", tag="d_ps",
                                    bufs=3)
                    col = e * 1024 + m * P
                    for k in range(CK):
                        nc.tensor.matmul(w1[:], rw1_sb[k][:, col:col + P],
                                         h2T[(j, k)][:],
                                         start=(k == 0), stop=(k == CK - 1))
                    gl = pd.tile([P, 512], BF16, name="gl", tag="gl", bufs=2)
                    nc.scalar.activation(gl[:], w1[:], ACT.Gelu)
                    hm = pd.tile([P, 512], BF16, name=f"hm_{j}_{e}_{m}",
                                 tag=f"hm{e * CK + m}", bufs=1)
                    nc.vector.tensor_mul(hm[:], gl[:], gb[e][:])
                    hmid.append(hm)
            # fused output matmul
            for tt in range(4):
                for n in range(2):
                    o2 = pd_ps.tile([P, 512], F32, name="o2_ps", tag="d_ps",
                                    bufs=3)
                    for k in range(4):
                        nc.tensor.matmul(o2[:], hTs[k][:, tt * P:(tt + 1) * P],
                                         sw2_sb[k][:, n * 512:(n + 1) * 512],
                                         start=(k == 0), stop=False)
                    for k in range(16):
                        nc.tensor.matmul(o2[:],
                                         hmid[k][:, tt * P:(tt + 1) * P],
                                         rw2_sb[k][:, n * 512:(n + 1) * 512],
                                         start=False, stop=(k == 15))
                    mo = pd.tile([P, 512], BF16, name="mo", tag="mo", bufs=2)
                    nc.vector.tensor_copy(mo[:], o2[:])
                    nc.sync.dma_start(
                        rsmo_in[j][tt * P:(tt + 1) * P,
                                   n * 512:(n + 1) * 512], mo[:])
            nc.gpsimd.collective_compute(
                "ReduceScatter", OP.add, replica_groups=GROUPS_B,
                ins=[rsmo_in[j].opt()], outs=[rsmo_out[j].opt()])
            # final residual for own tile j
            mo_bf = pd.tile([P, C], BF16, name=f"mo_{j}", tag="fmo", bufs=1)
            nc.sync.dma_start(mo_bf[:], rsmo_out[j][:])
            mo32 = pd.tile([P, C], F32, name=f"mo32_{j}", tag="fmo32", bufs=2)
            nc.vector.tensor_add(mo32[:], x2[j][:], mo_bf[:])
            nc.sync.dma_start(io["out"][j * P:(j + 1) * P, :], mo32[:])

        chunk_d(0)
        alloc_h2T(2)
        phase_c(2)
        chunk_d(1)
        alloc_h2T(3)
        phase_c(3)
        chunk_d(2)
        chunk_d(3)


# =============================================================================
# host side
# =============================================================================

def _rope_tables():
    freqs = (1.0 / (THETA ** (np.arange(0, HD, 2, dtype=np.float64) / HD)))
    t = np.arange(T, dtype=np.float64)
    emb = np.outer(t, freqs)                                # [T, 32]
    cos = np.concatenate([np.cos(emb), np.cos(emb)], 1).T   # [64, T]
    sin = np.concatenate([np.sin(emb), np.sin(emb)], 1).T   # [64, T]
    return cos.astype(np.float32), sin.astype(np.float32)


def _shard_inputs(inp):
    bf = ml_dtypes.bfloat16
    f32 = np.float32
    x = np.asarray(inp["x"], f32)                # [B, T, C]
    t_emb = np.asarray(inp["t_emb"], f32)
    ada_cat = np.concatenate([np.asarray(inp["ada1_w"], f32),
                              np.asarray(inp["ada2_w"], f32)], 1)  # [C, 4096]
    adab_cat = np.concatenate([np.asarray(inp["ada1_b"], f32),
                               np.asarray(inp["ada2_b"], f32)])    # [4096]
    wq = np.asarray(inp["wq"], f32)
    wk = np.asarray(inp["wk"], f32)
    wv = np.asarray(inp["wv"], f32)
    wo = np.asarray(inp["wo"], f32)
    sw1 = np.asarray(inp["sw1"], f32)
    sw3 = np.asarray(inp["sw3"], f32)
    sw2 = np.asarray(inp["sw2"], f32)
    rw1 = np.asarray(inp["re_w1"], f32)
    rw2 = np.asarray(inp["re_w2"], f32)
    rtw = np.asarray(inp["router_w"], f32)
    rtb = np.asarray(inp["router_bias"], f32)
    cosq, sinq = _rope_tables()
    ident = np.eye(P, dtype=f32)
    rotp = np.zeros((64, 64), dtype=f32)
    for i in range(32):
        rotp[32 + i, i] = -1.0     # out[p<32] = -q[p+32]
        rotp[i, 32 + i] = 1.0      # out[p>=32] = q[p-32]

    in_maps = []
    for c in range(NCORE):
        b, g = c // 4, c % 4
        xq = np.concatenate(
            [x[b, (4 * j + g) * P:(4 * j + g + 1) * P] for j in range(NT_Q)])
        m = {
            "x_q": np.ascontiguousarray(xq),
            "x_full": np.ascontiguousarray(x[b]),
            "temb_b": np.ascontiguousarray(t_emb[b].reshape(C, 1)),
            "ada_w_s": np.ascontiguousarray(
                ada_cat[:, g * 1024:(g + 1) * 1024]),
            "ada_b_s": np.ascontiguousarray(
                adab_cat[g * 1024:(g + 1) * 1024].reshape(1, 1024)),
            "wq_s": np.ascontiguousarray(
                wq[:, 256 * g:256 * (g + 1)]).astype(bf),
            "wkv_s": np.ascontiguousarray(np.concatenate(
                [wk[:, 64 * g:64 * (g + 1)],
                 wv[:, 64 * g:64 * (g + 1)]], 1)).astype(bf),
            "wo_s": np.ascontiguousarray(
                wo[256 * g:256 * (g + 1), :]).astype(bf),
            "cosq": cosq,
            "sinq": sinq,
            "identf": ident,
            "rotp": rotp.astype(bf),
            "swA_s": np.ascontiguousarray(np.concatenate(
                [sw1[:, 512 * g:512 * (g + 1)],
                 sw3[:, 512 * g:512 * (g + 1)]], 1)).astype(bf),
            "sw2_s": np.ascontiguousarray(
                sw2[512 * g:512 * (g + 1), :]).astype(bf),
            "rw1_e": np.ascontiguousarray(np.concatenate(
                [rw1[2 * g], rw1[2 * g + 1]], 1)).astype(bf),
            "rw2_e": np.ascontiguousarray(np.concatenate(
                [rw2[2 * g], rw2[2 * g + 1]], 0)).astype(bf),
            "router_w": rtw,
            "router_bias": rtb.reshape(1, E),
            "esel2": np.ascontiguousarray(
                np.eye(E, dtype=f32)[:, [2 * g, 2 * g + 1]]),
        }
        in_maps.append(m)
    return in_maps


_NC_CACHE = []


def _install_ntff_hook():
    """Provide antenv.axon_hooks (absent in this image) so trace=True works."""
    import sys
    import types
    try:
        import antenv
        if "antenv.axon_hooks" not in sys.modules:
            mod = types.ModuleType("antenv.axon_hooks")
            mod._hook = None

            def set_axon_ntff_profile_hook(h):
                mod._hook = h

            def get_axon_ntff_profile_hook():
                return mod._hook

            mod.set_axon_ntff_profile_hook = set_axon_ntff_profile_hook
            mod.get_axon_ntff_profile_hook = get_axon_ntff_profile_hook
            sys.modules["antenv.axon_hooks"] = mod
            antenv.axon_hooks = mod
        mod = sys.modules["antenv.axon_hooks"]
        if mod.get_axon_ntff_profile_hook() is None:
            from trn_agent_boot.trn_boot import _ntff_profile_via_ctypes
            hook = _ntff_profile_via_ctypes("/opt/axon/libaxon_pjrt.so")
            if hook is not None:
                mod.set_axon_ntff_profile_hook(hook)
        import concourse.bass_utils as bu
        bu.upload_artifacts = lambda d: d
        return True
    except Exception:
        return False


def kernel(**inputs):
    global LAST_EXEC_NS
    if not _NC_CACHE:
        _NC_CACHE.append(build_program())
    nc = _NC_CACHE[0]
    in_maps = _shard_inputs(inputs)
    trace = bool(int(os.environ.get("KB_TRACE", "0")))
    if trace:
        trace = _install_ntff_hook()
    res = None
    if trace:
        try:
            res = run_bass_kernel_spmd(nc, in_maps,
                                       core_ids=list(range(NCORE)),
                                       trace=True,
                                       tmpdir=os.environ.get("KB_TRACE_DIR"))
        except Exception as e:
            print(f"traced run failed ({e!r}); falling back to untraced")
            res = None
    if res is None:
        res = run_bass_kernel_spmd(nc, in_maps, core_ids=list(range(NCORE)))
    LAST_EXEC_NS = res.exec_time_ns
    out = np.empty((B, T, C), np.float32)
    for c in range(NCORE):
        b, g = c // 4, c % 4
        oc = res.results[c]["out"].astype(np.float32)
        for j in range(NT_Q):
            out[b, (4 * j + g) * P:(4 * j + g + 1) * P] = \
                oc[j * P:(j + 1) * P]
    return out


# revision 23
# speedup vs baseline: 1.0966x; 1.0966x over previous
"""nn_Block_67173288509603 on 8 TRN2 NeuronCores via Bass/Tile.

adaLN -> GQA block-causal attention (+RoPE) -> adaLN -> MoE (shared + top2-of-8).

v3: bf16 matmul paths, slab-pipelined chunked collectives, batch-local MoE.

Sharding (single SPMD program; per-core differences flow through inputs and
replica-group semantics):
  core c in 0..7, b = c//4 (batch), g = c%4 (kv-head group),
  experts {2g, 2g+1} for batch b, shared-expert hidden slice g of 4.
  Token ownership is INTERLEAVED: core c owns token tiles {4j+g : j=0..3} of
  batch b (tile = 128 tokens).  Slab j = global tokens [j*512,(j+1)*512).

  - Phase A: ada GEMV (fp32, 4-way sharded + tiny AllGather; scale/shift kept
    as per-C-dim columns), LN1 stats, per-owned-tile transpose of the
    normalized x with fused scale/shift applied on the transposed tiles,
    4 chunked AllGathers (bf16) of h1^T.
  - Phase B (per slab j): QKV projection (bf16) + RoPE (rot-half via +-1
    permutation matmul), block-causal attention for 4 q-heads/1 kv-head in
    bf16 (exp trick, fused denominator row), output projection partial,
    chunked ReduceScatter (own tile) + AllGather (full slab) over the batch
    group, both bf16.
  - Phase C (per slab j): own-tile residual for the final output; uniformly
    recompute x2/LN2 for all 4 tiles of the slab from x_full + gathered
    attention (bit-identical across the batch group), transpose into local
    h2^T (bf16), fp32 router logits + exact top-2 gates; per-core expert-pair
    gate rows selected by a one-hot matmul, bounced via DRAM for broadcast.
    No collectives.
  - Phase D (per chunk j = slab j): shared expert (hidden 1/4) + expert pair
    {2g, 2g+1} dense-masked, fused PSUM accumulation, chunked ReduceScatter
    over the batch group, final residual.
Output: each core returns its [512, 1024] interleaved quarter; host scatters.
"""

import os
import numpy as np
import ml_dtypes

import concourse.bass as bass
import concourse.mybir as mybir
import concourse.tile as tile
from concourse import bacc
from concourse.bass_utils import run_bass_kernel_spmd

F32 = mybir.dt.float32
BF16 = mybir.dt.bfloat16
AX = mybir.AxisListType
OP = mybir.AluOpType
ACT = mybir.ActivationFunctionType

B, T, C = 2, 2048, 1024
H, KVH, HD = 16, 4, 64
BLK = 128
THETA = 10000.0
E, TOPK = 8, 2
EPS_LN = 1e-5
P = 128
NCORE = 8
TQ = 512              # tokens per core quarter / slab size
NT_Q = TQ // P        # 4 owned tiles / slabs
CK = C // P           # 8 contraction tiles over C
NKB = T // BLK        # 16 kv blocks

LAST_EXEC_NS = None

GROUPS_B = [[0, 1, 2, 3], [4, 5, 6, 7]]


def build_program():
    nc = bacc.Bacc("TRN2", target_bir_lowering=False, debug=False,
                   num_devices=NCORE)

    def din(name, shape, dt):
        return nc.dram_tensor(name, list(shape), dt, kind="ExternalInput").ap()

    io = dict(
        x_q=din("x_q", [TQ, C], F32),
        x_full=din("x_full", [T, C], F32),
        temb_b=din("temb_b", [C, 1], F32),
        ada_w_s=din("ada_w_s", [C, 1024], F32),
        ada_b_s=din("ada_b_s", [1, 1024], F32),
        wq_s=din("wq_s", [C, 256], BF16),
        wkv_s=din("wkv_s", [C, 128], BF16),
        wo_s=din("wo_s", [256, C], BF16),
        cosq=din("cosq", [64, T], F32),
        sinq=din("sinq", [64, T], F32),
        identf=din("identf", [P, P], F32),
        rotp=din("rotp", [64, 64], BF16),
        swA_s=din("swA_s", [C, 1024], BF16),
        sw2_s=din("sw2_s", [512, C], BF16),
        rw1_e=din("rw1_e", [C, 2048], BF16),
        rw2_e=din("rw2_e", [2048, C], BF16),
        router_w=din("router_w", [C, E], F32),
        router_bias=din("router_bias", [1, E], F32),
        esel2=din("esel2", [E, 2], F32),
        out=nc.dram_tensor("out", [TQ, C], F32, kind="ExternalOutput").ap(),
    )

    with tile.TileContext(nc) as tc:
        _build(tc, io)
    nc.compile()
    return nc


def _build(tc, io):
    nc = tc.nc
    from contextlib import ExitStack
    PH = os.environ.get("KB_PHASES", "FULL")
    F_TTR = os.environ.get("KB_TTR", "0") == "1"  # InstTensorTensorReduce hangs this device
    F_VRECIP = os.environ.get("KB_VRECIP", "1") == "1"
    F_TS2 = os.environ.get("KB_TS2", "1") == "1"

    top = ExitStack()
    with top:
        dram = top.enter_context(tc.tile_pool(name="dram", bufs=1, space="DRAM"))
        pers = top.enter_context(tc.tile_pool(name="pers", bufs=1))

        # ---- chunked collective + scratch DRAM buffers --------------------
        ag_ada_in = dram.tile([1, 1024], F32, name="ag_ada_in")
        ag_ada_out = dram.tile([4, 1024], F32, name="ag_ada_out")
        h1ag_in = [dram.tile([C, P], BF16, name=f"h1ag_in_{j}")
                   for j in range(NT_Q)]
        h1ag_out = [dram.tile([4 * C, P], BF16, name=f"h1ag_out_{j}")
                    for j in range(NT_Q)]
        rsat_in = [dram.tile([TQ, C], BF16, name=f"rsat_in_{j}")
                   for j in range(NT_Q)]
        rsat_out = [dram.tile([P, C], BF16, name=f"rsat_out_{j}")
                    for j in range(NT_Q)]
        atag_in = [dram.tile([P, C], BF16, name=f"atag_in_{j}")
                   for j in range(NT_Q)]
        atag_out = [dram.tile([TQ, C], BF16, name=f"atag_out_{j}")
                    for j in range(NT_Q)]
        g2 = [dram.tile([2, TQ], BF16, name=f"g2_{j}") for j in range(NT_Q)]
        rsmo_in = [dram.tile([TQ, C], BF16, name=f"rsmo_in_{j}")
                   for j in range(NT_Q)]
        rsmo_out = [dram.tile([P, C], BF16, name=f"rsmo_out_{j}")
                    for j in range(NT_Q)]

        # ---- whole-kernel persistents (loads first: startup critical) -----
        x2 = [pers.tile([P, C], F32, name=f"x2_{t}", tag=f"x2_{t}")
              for t in range(NT_Q)]
        for t in range(NT_Q):
            nc.sync.dma_start(x2[t][:], io["x_q"][t * P:(t + 1) * P, :])
        identf = pers.tile([P, P], F32, name="identf", tag="identf")
        nc.sync.dma_start(identf[:], io["identf"][:])
        rotp = pers.tile([64, 64], BF16, name="rotp", tag="rotp")
        nc.sync.dma_start(rotp[:], io["rotp"][:])
        # ada scale/shift as per-C-dim columns: [P, 32], col a*8+k
        scsh = pers.tile([P, 32], F32, name="scsh", tag="scsh")
        ones1 = pers.tile([1, 64], BF16, name="ones1", tag="ones1")
        nc.vector.memset(ones1[:], 1.0)
        zl = pers.tile([1, 65], BF16, name="zl", tag="zl")
        nc.vector.memset(zl[:], 0.0)
        zr = pers.tile([1, 512], BF16, name="zr", tag="zr")
        nc.vector.memset(zr[:], 0.0)
        bias_bc = pers.tile([P, E], F32, name="bias_bc", tag="bias_bc")
        nc.sync.dma_start(
            bias_bc[:], io["router_bias"][0, :][None, :].to_broadcast([P, E]))
        esel2 = pers.tile([E, 2], F32, name="esel2", tag="esel2")
        nc.sync.dma_start(esel2[:], io["esel2"][:])
        rtw = [pers.tile([P, E], F32, name=f"rtw_{k}", tag=f"rtw{k}")
               for k in range(CK)]
        for k in range(CK):
            nc.sync.dma_start(rtw[k][:], io["router_w"][k * P:(k + 1) * P, :])

        # =====================================================================
        # Phase A
        # =====================================================================
        pa_stack = ExitStack()
        pa = pa_stack.enter_context(tc.tile_pool(name="pa", bufs=1))
        pa_ps = pa_stack.enter_context(tc.tile_pool(name="pa_ps", bufs=2,
                                                    space="PSUM"))

        # ada GEMV (fp32, own 1024-slice) + AllGather over batch group
        temb_sb = []
        adaw_sb = []
        for k in range(CK):
            tt = pa.tile([P, 1], F32, name=f"temb_{k}", tag=f"temb{k}")
            nc.sync.dma_start(tt[:], io["temb_b"][k * P:(k + 1) * P, :])
            temb_sb.append(tt)
            wt = pa.tile([P, 1024], F32, name=f"adaw_{k}", tag=f"adaw{k}")
            nc.sync.dma_start(wt[:], io["ada_w_s"][k * P:(k + 1) * P, :])
            adaw_sb.append(wt)
        adab_sb = pa.tile([1, 1024], F32, name="adab_sb", tag="adab_sb")
        nc.sync.dma_start(adab_sb[:], io["ada_b_s"][:])
        ada_sb = pa.tile([1, 1024], F32, name="ada_sb", tag="ada_sb")
        for n in range(2):
            ps = pa_ps.tile([1, 512], F32, name="ada_ps", tag="ada_ps", bufs=2)
            for k in range(CK):
                nc.tensor.matmul(ps[:], temb_sb[k][:],
                                 adaw_sb[k][:, n * 512:(n + 1) * 512],
                                 start=(k == 0), stop=(k == CK - 1))
            nc.vector.tensor_add(ada_sb[:, n * 512:(n + 1) * 512], ps[:],
                                 adab_sb[:, n * 512:(n + 1) * 512])
        nc.sync.dma_start(ag_ada_in[:], ada_sb[:])
        nc.gpsimd.collective_compute(
            "AllGather", OP.bypass, replica_groups=GROUPS_B,
            ins=[ag_ada_in.opt()], outs=[ag_ada_out.opt()])
        nc.sync.dma_start(
            scsh[:], ag_ada_out[:].rearrange("a (k p) -> p (a k)", p=P))
        nc.vector.tensor_scalar_add(scsh[:, 0:8], scsh[:, 0:8], 1.0)
        nc.vector.tensor_scalar_add(scsh[:, 16:24], scsh[:, 16:24], 1.0)
        sc1c, sh1c = scsh[:, 0:8], scsh[:, 8:16]
        sc2c, sh2c = scsh[:, 16:24], scsh[:, 24:32]

        def ln_stats(pool, x_sb, var4, mean4, col, name, sq_out=None):
            """mean/var stats of x_sb into column col of packed [P,4] tiles.

            var = E[x^2] - mean^2 (uncentered; fine for |x|~1, mean~0.03)."""
            s1 = pool.tile([P, 1], F32, name=f"{name}_s1", tag="ln_s1", bufs=2)
            nc.vector.tensor_reduce(s1[:], x_sb[:], axis=AX.X, op=OP.add)
            nc.vector.tensor_scalar_mul(mean4[:, col:col + 1], s1[:], 1.0 / C)
            if sq_out is None:
                sq = pool.tile([P, C], F32, name=f"{name}_sq", tag="ln_sq",
                               bufs=1)
            else:
                sq = sq_out
            ssq = pool.tile([P, 1], F32, name=f"{name}_ssq", tag="ln_ssq",
                            bufs=2)
            if F_TTR:
                nc.vector.tensor_tensor_reduce(sq[:], x_sb[:], x_sb[:], 1.0,
                                               0.0, op0=OP.mult, op1=OP.add,
                                               accum_out=ssq[:])
            else:
                nc.scalar.activation(sq[:], x_sb[:], ACT.Square,
                                     accum_out=ssq[:])
            nc.vector.tensor_scalar(var4[:, col:col + 1], ssq[:], 1.0 / C,
                                    EPS_LN, op0=OP.mult, op1=OP.add)

        def ln_finish4(pool, var4, mean4, name):
            """rstd4 = 1/sqrt(var4 - mean4^2); mrs4 = mean4*rstd4."""
            m2 = pool.tile([P, 4], F32, name=f"{name}_m2", tag="ln_m2",
                           bufs=2)
            nc.vector.tensor_mul(m2[:], mean4[:], mean4[:])
            nc.vector.tensor_sub(var4[:], var4[:], m2[:])
            rstd4 = pool.tile([P, 4], F32, name=f"{name}_rstd", tag="ln_rstd",
                              bufs=2)
            if F_VRECIP:
                std4 = pool.tile([P, 4], F32, name=f"{name}_std",
                                 tag="ln_std", bufs=2)
                nc.scalar.activation(std4[:], var4[:], ACT.Sqrt)
                nc.vector.reciprocal(rstd4[:], std4[:])
            else:
                t4 = pool.tile([P, 4], F32, name=f"{name}_lnr", tag="ln_std",
                               bufs=2)
                nc.scalar.activation(t4[:], var4[:], ACT.Ln)
                nc.scalar.activation(rstd4[:], t4[:], ACT.Exp, scale=-0.5)
            mrs4 = pool.tile([P, 4], F32, name=f"{name}_mrs", tag="ln_mrs",
                             bufs=2)
            nc.vector.tensor_mul(mrs4[:], mean4[:], rstd4[:])
            return rstd4, mrs4

        # LN1: packed stats for all tiles (one Sqrt table load), then
        # normalize + transpose + scale/shift on transposed tiles + AG chunks
        var4a = pa.tile([P, 4], F32, name="var4a", tag="ln_v4")
        mean4a = pa.tile([P, 4], F32, name="mean4a", tag="ln_mn4")
        for t in range(NT_Q):
            ln_stats(pa, x2[t], var4a, mean4a, t, f"l1_{t}")
        rstd1, mrs1 = ln_finish4(pa, var4a, mean4a, "l1")
        for t in range(NT_Q):
            hn = pa.tile([P, C], F32, name=f"hn_{t}", tag="hn", bufs=2)
            if F_TS2:
                nc.vector.tensor_scalar(hn[:], x2[t][:], rstd1[:, t:t + 1],
                                        mrs1[:, t:t + 1],
                                        op0=OP.mult, op1=OP.subtract)
            else:
                nc.vector.tensor_scalar_mul(hn[:], x2[t][:], rstd1[:, t:t + 1])
                nc.vector.tensor_scalar_sub(hn[:], hn[:], mrs1[:, t:t + 1])
            for k in range(CK):
                tp = pa_ps.tile([P, P], F32, name="h1t_ps", tag="h1t_ps",
                                bufs=2)
                nc.tensor.transpose(tp[:], hn[:, k * P:(k + 1) * P], identf[:])
                stg = pa.tile([P, P], BF16, name=f"h1t_{t}_{k}", tag="h1t",
                              bufs=3)
                nc.vector.tensor_copy(stg[:], tp[:])
                nc.sync.dma_start(h1ag_in[t][k * P:(k + 1) * P, :], stg[:])
            nc.gpsimd.collective_compute(
                "AllGather", OP.bypass, replica_groups=GROUPS_B,
                ins=[h1ag_in[t].opt()], outs=[h1ag_out[t].opt()])
        pa_stack.close()

        def dummy_out():
            with tc.tile_pool(name="dz", bufs=1) as pz:
                for t in range(NT_Q):
                    z = pz.tile([P, C], F32, name=f"dz_{t}", tag="dz", bufs=2)
                    nc.vector.memset(z[:], 0.0)
                    nc.sync.dma_start(io["out"][t * P:(t + 1) * P, :], z[:])
        if PH == "A":
            dummy_out()
            return

        # =====================================================================
        # pools for B/C/D
        # =====================================================================
        pc = top.enter_context(tc.tile_pool(name="pc", bufs=1))
        pc_ps = top.enter_context(tc.tile_pool(name="pc_ps", bufs=2,
                                               space="PSUM"))
        ph2 = top.enter_context(tc.tile_pool(name="ph2", bufs=1))
        h2T = {}  # (j, k) -> tile [P, TQ] bf16, local h2^T columns of slab j

        def phase_c(j):
            """own-tile residual; uniform slab LN2 -> h2T + gates (no comms)."""
            at_bf = pc.tile([P, C], BF16, name=f"at_{j}", tag="at", bufs=1)
            nc.sync.dma_start(at_bf[:], rsat_out[j][:])
            nc.vector.tensor_add(x2[j][:], x2[j][:], at_bf[:])
            # uniform path over the slab's 4 tiles
            var4 = pc.tile([P, 4], F32, name=f"var4_{j}", tag="ln_v4",
                           bufs=2)
            mean4 = pc.tile([P, 4], F32, name=f"mean4_{j}", tag="ln_mn4",
                            bufs=2)
            xfs, hns = [], []
            for r in range(4):
                row = j * TQ + r * P
                xf = pc.tile([P, C], F32, name=f"xf_{j}_{r}", tag="xf",
                             bufs=4)
                nc.sync.dma_start(xf[:], io["x_full"][row:row + P, :])
                ab = pc.tile([P, C], BF16, name=f"ab_{j}_{r}", tag="ab",
                             bufs=1)
                nc.sync.dma_start(ab[:], atag_out[j][r * P:(r + 1) * P, :])
                nc.vector.tensor_add(xf[:], xf[:], ab[:])
                hn = pc.tile([P, C], F32, name=f"hn2_{j}_{r}", tag=f"hn2{r}",
                             bufs=1)
                ln_stats(pc, xf, var4, mean4, r, f"l2_{j}_{r}", sq_out=hn)
                xfs.append(xf)
                hns.append(hn)
            rstd4, mrs4 = ln_finish4(pc, var4, mean4, f"l2_{j}")
            for r in range(4):
                nc.vector.tensor_scalar(hns[r][:], xfs[r][:],
                                        rstd4[:, r:r + 1], mrs4[:, r:r + 1],
                                        op0=OP.mult, op1=OP.subtract)
            # k-outer: transpose 4 tiles into one [P,512] psum, apply
            # scale/shift once into bf16 h2T + once into f32 for the router
            lg_sb = pc.tile([E, TQ], F32, name=f"lg_{j}", tag="lg", bufs=1)
            for k in range(CK):
                tp = pc_ps.tile([P, 512], F32, name="h2t_ps", tag="c_ps",
                                bufs=2)
                for r in range(4):
                    nc.tensor.transpose(tp[:, r * P:(r + 1) * P],
                                        hns[r][:, k * P:(k + 1) * P],
                                        identf[:])
                nc.vector.tensor_scalar(h2T[(j, k)][:], tp[:],
                                        sc2c[:, k:k + 1], sh2c[:, k:k + 1],
                                        op0=OP.mult, op1=OP.add)
                s32 = pc.tile([P, 512], F32, name=f"h2s_{j}_{k}",
                              tag="h2s32", bufs=1)
                nc.vector.tensor_scalar(s32[:], tp[:], sc2c[:, k:k + 1],
                                        sh2c[:, k:k + 1],
                                        op0=OP.mult, op1=OP.add)
                lgp = pc_ps.tile([P, 512], F32, name="lgp_ps", tag="c_ps",
                                 bufs=2)
                nc.tensor.matmul(lgp[0:E, :], rtw[k][:], s32[:],
                                 start=True, stop=True)
                lgs = pc.tile([E, TQ], F32, name="lgs", tag="lgs", bufs=1)
                nc.vector.tensor_copy(lgs[:], lgp[0:E, :])
                if k == 0:
                    nc.vector.tensor_copy(lg_sb[:], lgs[:])
                else:
                    nc.vector.tensor_add(lg_sb[:], lg_sb[:], lgs[:])
            # gates for the 4 tiles (batched sigmoid)
            sels = []
            for r in range(4):
                tp2 = pc_ps.tile([P, 512], F32, name="lgT_ps", tag="c_ps",
                                 bufs=2)
                nc.tensor.transpose(tp2[:, 0:E],
                                    lg_sb[:, r * P:(r + 1) * P],
                                    identf[0:E, 0:E])
                s_sb = pc.tile([P, E], F32, name=f"s_{j}_{r}", tag=f"s{r}",
                               bufs=1)
                nc.scalar.activation(s_sb[:], tp2[:, 0:E], ACT.Sigmoid)
                sels.append(s_sb)
            for r in range(4):
                s_sb = sels[r]
                sel = pc.tile([P, E], F32, name=f"sel_{j}_{r}", tag="sel",
                              bufs=2)
                nc.vector.tensor_add(sel[:], s_sb[:], bias_bc[:])
                m8 = pc.tile([P, 8], F32, name=f"m8_{j}_{r}", tag="m8",
                             bufs=2)
                nc.vector.max(m8[:], sel[:])
                mask = pc.tile([P, E], F32, name=f"mask_{j}_{r}", tag="mask",
                               bufs=2)
                nc.vector.tensor_scalar(mask[:], sel[:], m8[:, 1:2], None,
                                        op0=OP.is_ge)
                sm = pc.tile([P, E], F32, name=f"sm_{j}_{r}", tag="sm",
                             bufs=2)
                nc.vector.tensor_mul(sm[:], s_sb[:], mask[:])
                den = pc.tile([P, 1], F32, name=f"den_{j}_{r}", tag="den",
                              bufs=2)
                nc.vector.tensor_reduce(den[:], sm[:], axis=AX.X, op=OP.add)
                nc.vector.tensor_scalar_add(den[:], den[:], 1e-9)
                rden = pc.tile([P, 1], F32, name=f"rden_{j}_{r}", tag="rden",
                               bufs=2)
                nc.vector.reciprocal(rden[:], den[:])
                ge = pc.tile([P, E], F32, name=f"ge_{j}_{r}", tag="ge",
                             bufs=2)
                nc.vector.tensor_scalar_mul(ge[:], sm[:], rden[:])
                gT_ps = pc_ps.tile([P, 512], F32, name="gT_ps", tag="c_ps",
                                   bufs=2)
                nc.tensor.transpose(gT_ps[0:E, 0:P], ge[:], identf[:])
                geT = pc.tile([E, P], F32, name=f"geT_{j}_{r}", tag="geT",
                              bufs=1)
                nc.vector.tensor_copy(geT[:], gT_ps[0:E, 0:P])
                sel_ps = pc_ps.tile([P, 512], F32, name="sel_ps", tag="c_ps",
                                    bufs=2)
                nc.tensor.matmul(sel_ps[0:2, 0:P], esel2[:], geT[:],
                                 start=True, stop=True)
                sel_bf = pc.tile([2, P], BF16, name=f"selb_{j}_{r}",
                                 tag="selb", bufs=1)
                nc.vector.tensor_copy(sel_bf[:], sel_ps[0:2, 0:P])
                nc.sync.dma_start(g2[j][:, r * P:(r + 1) * P], sel_bf[:])

        # ---------------- Phase B ----------------
        pb = ExitStack()
        pB = pb.enter_context(tc.tile_pool(name="pB", bufs=1))
        pb_mm = pb.enter_context(tc.tile_pool(name="pb_mm", bufs=2,
                                              space="PSUM"))
        pb_st = pb.enter_context(tc.tile_pool(name="pb_st", bufs=2,
                                              space="PSUM"))
        pb_yt = pb.enter_context(tc.tile_pool(name="pb_yt", bufs=2,
                                              space="PSUM"))
        cosq = pB.tile([64, T], F32, name="cosq", tag="cosq")
        sinq = pB.tile([64, T], F32, name="sinq", tag="sinq")
        nc.sync.dma_start(cosq[:], io["cosq"][:])
        nc.sync.dma_start(sinq[:], io["sinq"][:])
        wq_sb, wkv_sb = [], []
        for k in range(CK):
            w = pB.tile([P, 256], BF16, name=f"wq_{k}", tag=f"wq{k}")
            nc.sync.dma_start(w[:], io["wq_s"][k * P:(k + 1) * P, :])
            wq_sb.append(w)
            w = pB.tile([P, P], BF16, name=f"wkv_{k}", tag=f"wkv{k}")
            nc.sync.dma_start(w[:], io["wkv_s"][k * P:(k + 1) * P, :])
            wkv_sb.append(w)
        wo_sb = []
        for k in range(2):
            w = pB.tile([P, C], BF16, name=f"wo_{k}", tag=f"wo{k}")
            nc.sync.dma_start(w[:], io["wo_s"][k * P:(k + 1) * P, :])
            wo_sb.append(w)
        krope = pB.tile([64, T], BF16, name="krope", tag="krope")
        vav = [pB.tile([P, 65], BF16, name=f"vav_{kb}", tag=f"vav{kb}")
               for kb in range(NKB)]
        for kb in range(NKB):
            nc.vector.memset(vav[kb][:, 64:65], 1.0)

        def slab_b(j):
            js = j * 512
            h1c = []
            for k in range(CK):
                hc = pB.tile([P, 512], BF16, name=f"h1c_{j}_{k}",
                             tag=f"h1c{k}", bufs=2)
                for r in range(4):
                    nc.sync.dma_start(
                        hc[:, r * P:(r + 1) * P],
                        h1ag_out[j][r * C + k * P: r * C + (k + 1) * P, :])
                nc.vector.tensor_scalar(hc[:], hc[:], sc1c[:, k:k + 1],
                                        sh1c[:, k:k + 1],
                                        op0=OP.mult, op1=OP.add)
                h1c.append(hc)
            # K projection + rope
            kv_ps = pb_mm.tile([P, 512], F32, name="kv_ps", tag="mm", bufs=2)
            for k in range(CK):
                nc.tensor.matmul(kv_ps[0:64, :], wkv_sb[k][:, 0:64],
                                 h1c[k][:], start=(k == 0), stop=(k == CK - 1))
            kt = pB.tile([64, 512], BF16, name="kt", tag="kt", bufs=2)
            nc.vector.tensor_copy(kt[:], kv_ps[0:64, :])
            krot_ps = pb_mm.tile([P, 512], F32, name="krot_ps", tag="mm",
                                 bufs=2)
            nc.tensor.matmul(krot_ps[0:64, :], rotp[:], kt[:],
                             start=True, stop=True)
            kc = pB.tile([64, 512], F32, name="kc", tag="kc", bufs=1)
            nc.vector.tensor_mul(kc[:], kt[:], cosq[:, js:js + 512])
            ks = pB.tile([64, 512], F32, name="ks", tag="ks", bufs=1)
            nc.vector.tensor_mul(ks[:], krot_ps[0:64, :],
                                 sinq[:, js:js + 512])
            nc.vector.tensor_add(krope[:, js:js + 512], kc[:], ks[:])
            # V^T directly: vav[4j+i][:, 0:64] = (h1c_block)^T @ wv
            for i in range(4):
                vav_ps = pb_mm.tile([P, 512], F32, name="vav_ps", tag="mm",
                                    bufs=2)
                for k in range(CK):
                    nc.tensor.matmul(vav_ps[:, 0:64],
                                     h1c[k][:, i * P:(i + 1) * P],
                                     wkv_sb[k][:, 64:128],
                                     start=(k == 0), stop=(k == CK - 1))
                nc.vector.tensor_copy(vav[4 * j + i][:, 0:64],
                                      vav_ps[:, 0:64])
            # Q projection + rope (4 heads)
            qrope = []
            for h in range(4):
                q_ps = pb_mm.tile([P, 512], F32, name="q_ps", tag="mm",
                                  bufs=2)
                for k in range(CK):
                    nc.tensor.matmul(q_ps[0:64, :],
                                     wq_sb[k][:, h * 64:(h + 1) * 64],
                                     h1c[k][:], start=(k == 0),
                                     stop=(k == CK - 1))
                qt = pB.tile([64, 512], BF16, name="qt", tag="qt", bufs=3)
                nc.vector.tensor_copy(qt[:], q_ps[0:64, :])
                qrot_ps = pb_mm.tile([P, 512], F32, name="qrot_ps", tag="mm",
                                     bufs=2)
                nc.tensor.matmul(qrot_ps[0:64, :], rotp[:], qt[:],
                                 start=True, stop=True)
                qc = pB.tile([64, 512], F32, name="qc", tag="qc", bufs=1)
                nc.vector.tensor_mul(qc[:], qt[:], cosq[:, js:js + 512])
                qs = pB.tile([64, 512], F32, name="qs", tag="qs", bufs=1)
                nc.vector.tensor_mul(qs[:], qrot_ps[0:64, :],
                                     sinq[:, js:js + 512])
                qr = pB.tile([64, 512], BF16, name=f"qr_{h}", tag=f"qr{h}",
                             bufs=2)
                nc.vector.tensor_add(qr[:], qc[:], qs[:])
                qrope.append(qr)
            # attention, two heads interleaved per kb (keeps PE fed while
            # the scalar engine computes the exp of the other head)
            ytall = [pB.tile([P, 512], BF16, name=f"ytall_{j}_{i}",
                             tag=f"ytall{i}", bufs=2) for i in range(2)]
            lr1 = pB.tile([1, 2048], F32, name=f"lr1_{j}", tag="lr1", bufs=1)
            ytcs = []
            for h0 in (0, 2):
                yts = [pb_yt.tile([65, 512], F32, name="yt_ps", tag="yt",
                                  bufs=2) for _ in range(2)]
                for kb in range(4 * j + 4):
                    qoff = max(0, (kb - 4 * j) * P)
                    w = 512 - qoff
                    for i in range(2):
                        st = pb_st.tile([P, 512], F32, name="st_ps", tag="st",
                                        bufs=2)
                        nc.tensor.matmul(st[:, :w],
                                         krope[:, kb * P:(kb + 1) * P],
                                         qrope[h0 + i][:, qoff:512],
                                         start=True, stop=True)
                        pexp = pB.tile([P, 512], BF16, name="pexp",
                                       tag="pexp", bufs=4)
                        nc.scalar.activation(pexp[:, :w], st[:, :w], ACT.Exp,
                                             scale=0.125)
                        nc.tensor.matmul(yts[i][:, qoff:512], vav[kb][:],
                                         pexp[:, :w], start=(kb == 0),
                                         stop=False)
                for i in range(2):
                    h = h0 + i
                    nc.tensor.matmul(yts[i][:], zl[:], zr[:], start=False,
                                     stop=True)
                    nc.vector.tensor_copy(lr1[0:1, h * 512:(h + 1) * 512],
                                          yts[i][64:65, :])
                    ytc = pB.tile([64, 512], BF16, name=f"ytc_{h}",
                                  tag=f"ytc{h}", bufs=2)
                    nc.vector.tensor_copy(ytc[:], yts[i][0:64, :])
                    ytcs.append(ytc)
            rec1 = pB.tile([1, 2048], F32, name=f"rec1_{j}", tag="rec1",
                           bufs=1)
            nc.vector.reciprocal(rec1[:], lr1[:])
            rec1b = pB.tile([1, 2048], BF16, name=f"rec1b_{j}", tag="rec1b",
                            bufs=1)
            nc.vector.tensor_copy(rec1b[:], rec1[:])
            for h in range(4):
                recb = pb_mm.tile([P, 512], F32, name="recb_ps", tag="mm",
                                  bufs=2)
                nc.tensor.matmul(recb[0:64, :], ones1[:],
                                 rec1b[0:1, h * 512:(h + 1) * 512],
                                 start=True, stop=True)
                nc.vector.tensor_mul(
                    ytall[h // 2][(h % 2) * 64:(h % 2) * 64 + 64, :],
                    ytcs[h][:], recb[0:64, :])
            # output projection + RS (own tile) + AG (full slab)
            for tt in range(4):
                for n in range(2):
                    o_ps = pb_mm.tile([P, 512], F32, name="o_ps", tag="mm",
                                      bufs=2)
                    for k in range(2):
                        nc.tensor.matmul(o_ps[:],
                                         ytall[k][:, tt * P:(tt + 1) * P],
                                         wo_sb[k][:, n * 512:(n + 1) * 512],
                                         start=(k == 0), stop=(k == 1))
                    o_bf = pB.tile([P, 512], BF16, name="o_bf", tag="o_bf",
                                   bufs=3)
                    nc.scalar.copy(o_bf[:], o_ps[:])
                    nc.sync.dma_start(
                        rsat_in[j][tt * P:(tt + 1) * P,
                                   n * 512:(n + 1) * 512], o_bf[:])
            nc.gpsimd.collective_compute(
                "ReduceScatter", OP.add, replica_groups=GROUPS_B,
                ins=[rsat_in[j].opt()], outs=[rsat_out[j].opt()])
            # bounce the RS output through SBUF into a fresh buffer: an AG
            # whose input aliases another collective's output lacks the
            # input-writer sync annotation and hangs the device.
            atb = pB.tile([P, C], BF16, name=f"atb_{j}", tag="atb", bufs=2)
            nc.sync.dma_start(atb[:], rsat_out[j][:])
            nc.sync.dma_start(atag_in[j][:], atb[:])
            nc.gpsimd.collective_compute(
                "AllGather", OP.bypass, replica_groups=GROUPS_B,
                ins=[atag_in[j].opt()], outs=[atag_out[j].opt()])

        def alloc_h2T(j):
            for k in range(CK):
                h2T[(j, k)] = ph2.tile([P, TQ], BF16, name=f"h2T_{j}_{k}",
                                       tag=f"h2T{k}", bufs=3)

        if PH in ("B", "C"):
            slab_b(0)
            slab_b(1)
            slab_b(2)
            slab_b(3)
            pb.close()
            if PH == "B":
                dummy_out()
                return
            for j in range(NT_Q):
                alloc_h2T(j)
                phase_c(j)
            dummy_out()
            return
        slab_b(0)
        slab_b(1)
        alloc_h2T(0)
        phase_c(0)
        slab_b(2)
        alloc_h2T(1)
        phase_c(1)
        slab_b(3)
        pb.close()

        # ---------------- Phase D ----------------
        prw = top.enter_context(tc.tile_pool(name="prw", bufs=1))
        pd = top.enter_context(tc.tile_pool(name="pd", bufs=1))
        pd_ps = top.enter_context(tc.tile_pool(name="pd_ps", bufs=3,
                                               space="PSUM"))
        swA_sb, rw1_sb = [], []
        for k in range(CK):
            w = prw.tile([P, 1024], BF16, name=f"swA_{k}", tag=f"swA{k}")
            nc.sync.dma_start(w[:], io["swA_s"][k * P:(k + 1) * P, :])
            swA_sb.append(w)
            w = prw.tile([P, 2048], BF16, name=f"rw1_{k}", tag=f"rw1{k}")
            nc.sync.dma_start(w[:], io["rw1_e"][k * P:(k + 1) * P, :])
            rw1_sb.append(w)
        sw2_sb = []
        for k in range(4):
            w = prw.tile([P, C], BF16, name=f"sw2_{k}", tag=f"sw2{k}")
            nc.sync.dma_start(w[:], io["sw2_s"][k * P:(k + 1) * P, :])
            sw2_sb.append(w)
        rw2_sb = []
        for k in range(16):
            w = prw.tile([P, C], BF16, name=f"rw2_{k}", tag=f"rw2{k}")
            nc.sync.dma_start(w[:], io["rw2_e"][k * P:(k + 1) * P, :])
            rw2_sb.append(w)

        def chunk_d(j):
            gb = []
            for e in range(2):
                g = pd.tile([P, TQ], BF16, name=f"gb_{j}_{e}", tag=f"gb{e}",
                            bufs=1)
                nc.sync.dma_start(
                    g[:], g2[j][e, :][None, :].to_broadcast([P, TQ]))
                gb.append(g)
            # shared expert (hidden slice g of 4 -> 512 dims local)
            hTs = []
            for m in range(4):
                a1 = pd_ps.tile([P, 512], F32, name="a1_ps", tag="d_ps",
                                bufs=3)
                for k in range(CK):
                    nc.tensor.matmul(a1[:], swA_sb[k][:, m * P:(m + 1) * P],
                                     h2T[(j, k)][:],
                                     start=(k == 0), stop=(k == CK - 1))
                stmp = pd.tile([P, 512], BF16, name="stmp", tag="stmp",
                               bufs=1)
                nc.scalar.activation(stmp[:], a1[:], ACT.Silu)
                a3 = pd_ps.tile([P, 512], F32, name="a3_ps", tag="d_ps",
                                bufs=3)
                for k in range(CK):
                    nc.tensor.matmul(
                        a3[:], swA_sb[k][:, 512 + m * P:512 + (m + 1) * P],
                        h2T[(j, k)][:], start=(k == 0), stop=(k == CK - 1))
                ht = pd.tile([P, 512], BF16, name=f"hTs_{j}_{m}",
                             tag=f"hTs{m}", bufs=1)
                nc.vector.tensor_mul(ht[:], stmp[:], a3[:])
                hTs.append(ht)
            # routed expert pair
            hmid = []
            for e in range(2):
                for m in range(CK):
                    w1 = pd_ps.tile([P, 512], F32, name="w1_ps", tag="d_ps",
                                    bufs=3)
                    col = e * 1024 + m * P
                    for k in range(CK):
                        nc.tensor.matmul(w1[:], rw1_sb[k][:, col:col + P],
                                         h2T[(j, k)][:],
                                         start=(k == 0), stop=(k == CK - 1))
                    gl = pd.tile([P, 512], BF16, name="gl", tag="gl", bufs=2)
                    nc.scalar.activation(gl[:], w1[:], ACT.Gelu)
                    hm = pd.tile([P, 512], BF16, name=f"hm_{j}_{e}_{m}",
                                 tag=f"hm{e * CK + m}", bufs=1)
                    nc.vector.tensor_mul(hm[:], gl[:], gb[e][:])
                    hmid.append(hm)
            # fused output matmul
            for tt in range(4):
                for n in range(2):
                    o2 = pd_ps.tile([P, 512], F32, name="o2_ps", tag="d_ps",
                                    bufs=3)
                    for k in range(4):
                        nc.tensor.matmul(o2[:], hTs[k][:, tt * P:(tt + 1) * P],
                                         sw2_sb[k][:, n * 512:(n + 1) * 512],
                                         start=(k == 0), stop=False)
                    for k in range(16):
                        nc.tensor.matmul(o2[:],
                                         hmid[k][:, tt * P:(tt + 1) * P],
                                         rw2_sb[k][:, n * 512:(n + 1) * 512],
                                         start=False, stop=(k == 15))
                    mo = pd.tile([P, 512], BF16, name="mo", tag="mo", bufs=2)
                    nc.scalar.copy(mo[:], o2[:])
                    nc.sync.dma_start(
                        rsmo_in[j][tt * P:(tt + 1) * P,
                                   n * 512:(n + 1) * 512], mo[:])
            nc.gpsimd.collective_compute(
                "ReduceScatter", OP.add, replica_groups=GROUPS_B,
                ins=[rsmo_in[j].opt()], outs=[rsmo_out[j].opt()])
            # final residual for own tile j
            mo_bf = pd.tile([P, C], BF16, name=f"mo_{j}", tag="fmo", bufs=1)
            nc.sync.dma_start(mo_bf[:], rsmo_out[j][:])
            mo32 = pd.tile([P, C], F32, name=f"mo32_{j}", tag="fmo32", bufs=1)
            nc.vector.tensor_add(mo32[:], x2[j][:], mo_bf[:])
            nc.sync.dma_start(io["out"][j * P:(j + 1) * P, :], mo32[:])

        alloc_h2T(2)
        phase_c(2)
        chunk_d(0)
        alloc_h2T(3)
        phase_c(3)
        chunk_d(1)
        chunk_d(2)
        chunk_d(3)


# =============================================================================
# host side
# =============================================================================

def _rope_tables():
    freqs = (1.0 / (THETA ** (np.arange(0, HD, 2, dtype=np.float64) / HD)))
    t = np.arange(T, dtype=np.float64)
    emb = np.outer(t, freqs)                                # [T, 32]
    cos = np.concatenate([np.cos(emb), np.cos(emb)], 1).T   # [64, T]
    sin = np.concatenate([np.sin(emb), np.sin(emb)], 1).T   # [64, T]
    return cos.astype(np.float32), sin.astype(np.float32)


def _shard_inputs(inp):
    bf = ml_dtypes.bfloat16
    f32 = np.float32
    x = np.asarray(inp["x"], f32)                # [B, T, C]
    t_emb = np.asarray(inp["t_emb"], f32)
    ada_cat = np.concatenate([np.asarray(inp["ada1_w"], f32),
                              np.asarray(inp["ada2_w"], f32)], 1)  # [C, 4096]
    adab_cat = np.concatenate([np.asarray(inp["ada1_b"], f32),
                               np.asarray(inp["ada2_b"], f32)])    # [4096]
    wq = np.asarray(inp["wq"], f32)
    wk = np.asarray(inp["wk"], f32)
    wv = np.asarray(inp["wv"], f32)
    wo = np.asarray(inp["wo"], f32)
    sw1 = np.asarray(inp["sw1"], f32)
    sw3 = np.asarray(inp["sw3"], f32)
    sw2 = np.asarray(inp["sw2"], f32)
    rw1 = np.asarray(inp["re_w1"], f32)
    rw2 = np.asarray(inp["re_w2"], f32)
    rtw = np.asarray(inp["router_w"], f32)
    rtb = np.asarray(inp["router_bias"], f32)
    cosq, sinq = _rope_tables()
    ident = np.eye(P, dtype=f32)
    rotp = np.zeros((64, 64), dtype=f32)
    for i in range(32):
        rotp[32 + i, i] = -1.0     # out[p<32] = -q[p+32]
        rotp[i, 32 + i] = 1.0      # out[p>=32] = q[p-32]

    in_maps = []
    for c in range(NCORE):
        b, g = c // 4, c % 4
        xq = np.concatenate(
            [x[b, (4 * j + g) * P:(4 * j + g + 1) * P] for j in range(NT_Q)])
        m = {
            "x_q": np.ascontiguousarray(xq),
            "x_full": np.ascontiguousarray(x[b]),
            "temb_b": np.ascontiguousarray(t_emb[b].reshape(C, 1)),
            "ada_w_s": np.ascontiguousarray(
                ada_cat[:, g * 1024:(g + 1) * 1024]),
            "ada_b_s": np.ascontiguousarray(
                adab_cat[g * 1024:(g + 1) * 1024].reshape(1, 1024)),
            "wq_s": np.ascontiguousarray(
                wq[:, 256 * g:256 * (g + 1)]).astype(bf),
            "wkv_s": np.ascontiguousarray(np.concatenate(
                [wk[:, 64 * g:64 * (g + 1)],
                 wv[:, 64 * g:64 * (g + 1)]], 1)).astype(bf),
            "wo_s": np.ascontiguousarray(
                wo[256 * g:256 * (g + 1), :]).astype(bf),
            "cosq": cosq,
            "sinq": sinq,
            "identf": ident,
            "rotp": rotp.astype(bf),
            "swA_s": np.ascontiguousarray(np.concatenate(
                [sw1[:, 512 * g:512 * (g + 1)],
                 sw3[:, 512 * g:512 * (g + 1)]], 1)).astype(bf),
            "sw2_s": np.ascontiguousarray(
                sw2[512 * g:512 * (g + 1), :]).astype(bf),
            "rw1_e": np.ascontiguousarray(np.concatenate(
                [rw1[2 * g], rw1[2 * g + 1]], 1)).astype(bf),
            "rw2_e": np.ascontiguousarray(np.concatenate(
                [rw2[2 * g], rw2[2 * g + 1]], 0)).astype(bf),
            "router_w": rtw,
            "router_bias": rtb.reshape(1, E),
            "esel2": np.ascontiguousarray(
                np.eye(E, dtype=f32)[:, [2 * g, 2 * g + 1]]),
        }
        in_maps.append(m)
    return in_maps


_NC_CACHE = []


def _install_ntff_hook():
    """Provide antenv.axon_hooks (absent in this image) so trace=True works."""
    import sys
    import types
    try:
        import antenv
        if "antenv.axon_hooks" not in sys.modules:
            mod = types.ModuleType("antenv.axon_hooks")
            mod._hook = None

            def set_axon_ntff_profile_hook(h):
                mod._hook = h

            def get_axon_ntff_profile_hook():
                return mod._hook

            mod.set_axon_ntff_profile_hook = set_axon_ntff_profile_hook
            mod.get_axon_ntff_profile_hook = get_axon_ntff_profile_hook
            sys.modules["antenv.axon_hooks"] = mod
            antenv.axon_hooks = mod
        mod = sys.modules["antenv.axon_hooks"]
        if mod.get_axon_ntff_profile_hook() is None:
            from trn_agent_boot.trn_boot import _ntff_profile_via_ctypes
            hook = _ntff_profile_via_ctypes("/opt/axon/libaxon_pjrt.so")
            if hook is not None:
                mod.set_axon_ntff_profile_hook(hook)
        import concourse.bass_utils as bu
        bu.upload_artifacts = lambda d: d
        return True
    except Exception:
        return False


def kernel(**inputs):
    global LAST_EXEC_NS
    if not _NC_CACHE:
        _NC_CACHE.append(build_program())
    nc = _NC_CACHE[0]
    in_maps = _shard_inputs(inputs)
    trace = bool(int(os.environ.get("KB_TRACE", "0")))
    if trace:
        trace = _install_ntff_hook()
    res = None
    if trace:
        try:
            res = run_bass_kernel_spmd(nc, in_maps,
                                       core_ids=list(range(NCORE)),
                                       trace=True,
                                       tmpdir=os.environ.get("KB_TRACE_DIR"))
        except Exception as e:
            print(f"traced run failed ({e!r}); falling back to untraced")
            res = None
    if res is None:
        res = run_bass_kernel_spmd(nc, in_maps, core_ids=list(range(NCORE)))
    LAST_EXEC_NS = res.exec_time_ns
    out = np.empty((B, T, C), np.float32)
    for c in range(NCORE):
        b, g = c // 4, c % 4
        oc = res.results[c]["out"].astype(np.float32)
        for j in range(NT_Q):
            out[b, (4 * j + g) * P:(4 * j + g + 1) * P] = \
                oc[j * P:(j + 1) * P]
    return out
